# revision 36
# baseline (speedup 1.0000x reference)
"""Trainium2 Bass kernel for BoundaryCycleEncoder.

Architecture notes:
- Pure data parallel: 64 samples sharded 8 per core across 8 NeuronCores.
- The reverse direction encode(flip(x)) flipped back equals running the same
  encoder with the prev/next blocks of W1 swapped, so no data flipping is done;
  both directions share the projected h0.
- Per layer, h is kept in two layouts (all matmul-path data in fp16):
    A: [H=128 partitions, L tokens] (with +-1 halo columns) feeds the matmuls
    B: [128 tokens, H] is where LayerNorm stats/normalization happen
  mlp1 = relu(W1 . [prev;self;next]) is computed weight-stationary into PSUM
  (layout A); mlp2 runs activation-stationary (lhsT = mlp1 128-token chunks)
  landing delta in layout B PSUM, where one identity-matmul accumulates the
  residual h so y = h + delta materializes directly in PSUM; bn_stats reads
  PSUM per 128-token block; normalize+relu is one ScalarE activation per block
  with per-partition (per-token) scale/bias; PE transposes bring h back to
  layout A. PSUM->SBUF evacuations alternate DVE/ACT to balance engines.
"""

import os
os.environ.setdefault("NEURON_RT_RESET_CORES", "1")  # recover from wedged cores

import numpy as np
import ml_dtypes

H = 128
NL = 4
EPS = 1e-5
CORES = 8
CH = 512           # tokens per chunk (1 PSUM bank at fp32)
NBLK = CH // 128   # 128-token blocks per chunk


def _build(BS, L, generic_b2=False, generic_gb=False, debug_taps=False):
    """Build the SPMD Bass program for one core holding BS samples of length L."""
    import concourse.bacc as bacc
    import concourse.mybir as mybir
    import concourse.tile as tile

    NCH = L // CH
    f32 = mybir.dt.float32
    f16 = mybir.dt.float16
    AF = mybir.ActivationFunctionType
    ALU = mybir.AluOpType

    nc = bacc.Bacc()

    tokens_d = nc.dram_tensor("tokens_sh", [BS, L, 8], f16, kind="ExternalInput")
    Wp_d = nc.dram_tensor("Wp", [8, H], f16, kind="ExternalInput")
    W1_d = nc.dram_tensor("W1h", [NL, 3 * H, H], f16, kind="ExternalInput")
    W2_d = nc.dram_tensor("W2h", [NL, H, H], f16, kind="ExternalInput")
    b1T_d = nc.dram_tensor("b1T", [H, NL], f32, kind="ExternalInput")
    bpT_d = nc.dram_tensor("bpT", [H, 1], f32, kind="ExternalInput")
    Wsbc_d = nc.dram_tensor("Ws_bc", [H, H], f32, kind="ExternalInput")
    id_d = nc.dram_tensor("id128h", [H, H], f16, kind="ExternalInput")
    ones_d = nc.dram_tensor("ones128", [H, 1], f32, kind="ExternalInput")
    if generic_b2:
        b2_d = nc.dram_tensor("b2bc", [H, NL * H], f32, kind="ExternalInput")
    if generic_gb:
        gb_d = nc.dram_tensor("gb_bc", [H, 2 * NL * H], f32, kind="ExternalInput")

    h_out_d = nc.dram_tensor("h_out", [BS, L, H], f32, kind="ExternalOutput")
    pooled_d = nc.dram_tensor("pooled_out", [BS, H], f32, kind="ExternalOutput")
    if debug_taps:
        dbg_h0_d = nc.dram_tensor("dbg_h0", [L, H], f16, kind="ExternalOutput")
        dbg_hB_d = nc.dram_tensor("dbg_hB", [NL, 2, L, H], f16, kind="ExternalOutput")

    with tile.TileContext(nc) as tc:
        con = tc.alloc_tile_pool(name="con", bufs=1)      # constants/weights
        hA = tc.alloc_tile_pool(name="hA", bufs=2)        # [128, L+2] layout-A h (f16)
        hB = tc.alloc_tile_pool(name="hB", bufs=2)        # [128, L/128, 128] layout-B h (f16)
        work = tc.alloc_tile_pool(name="work", bufs=4)    # per-chunk tiles
        stat = tc.alloc_tile_pool(name="stat", bufs=6)    # small stats tiles
        fin = tc.alloc_tile_pool(name="fin", bufs=2)      # final-stage tiles
        pmm = tc.alloc_tile_pool(name="pmm", bufs=2, space="PSUM")
        pyy = tc.alloc_tile_pool(name="pyy", bufs=3, space="PSUM")
        ptr = tc.alloc_tile_pool(name="ptr", bufs=2, space="PSUM")
        pus = tc.alloc_tile_pool(name="pus", bufs=1, space="PSUM")

        # ---- load constants ----
        W1_sb = con.tile([128, NL, 3, H], f16)
        nc.sync.dma_start(out=W1_sb, in_=W1_d[:].rearrange("nl (j p) h -> p nl j h", p=128))
        W2_sb = con.tile([128, NL, H], f16)
        nc.sync.dma_start(out=W2_sb, in_=W2_d[:].rearrange("nl p h -> p nl h"))
        Wp_sb = con.tile([8, H], f16)
        nc.sync.dma_start(out=Wp_sb, in_=Wp_d[:])
        b1T_sb = con.tile([128, NL], f32)
        nc.sync.dma_start(out=b1T_sb, in_=b1T_d[:])
        bpT_sb = con.tile([128, 1], f32)
        nc.sync.dma_start(out=bpT_sb, in_=bpT_d[:])
        Wsbc_sb = con.tile([128, H], f32)
        nc.sync.dma_start(out=Wsbc_sb, in_=Wsbc_d[:])
        id_sb = con.tile([128, H], f16)
        nc.sync.dma_start(out=id_sb, in_=id_d[:])
        ones_sb = con.tile([128, 1], f32)
        nc.sync.dma_start(out=ones_sb, in_=ones_d[:])
        if generic_b2:
            b2_sb = con.tile([128, NL, H], f32)
            nc.sync.dma_start(out=b2_sb, in_=b2_d[:].rearrange("p (nl h) -> p nl h", nl=NL))
        if generic_gb:
            gb_sb = con.tile([128, 2, NL, H], f32)
            nc.sync.dma_start(out=gb_sb, in_=gb_d[:].rearrange("p (t nl h) -> p t nl h", t=2, nl=NL))
        eps_sb = con.tile([128, 1], f32)
        nc.vector.memset(eps_sb, EPS)
        eps4_sb = con.tile([128, 1], f32)
        nc.vector.memset(eps4_sb, 4.0 * EPS)

        def evac(dst, src, eng):
            """PSUM->SBUF copy on the chosen engine ('v' DVE / 'a' ACT)."""
            if eng == "v":
                nc.vector.tensor_copy(dst, src)
            else:
                nc.scalar.activation(dst, src, AF.Copy)

        def transpose_chunk(src_B, dst_A, c, eng):
            """PE-transpose 4 blocks of layout-B h into layout-A columns (+halo)."""
            tps = ptr.tile([128, CH], f16, tag="tps", name="tps")
            for k in range(NBLK):
                nc.tensor.transpose(tps[:, k * 128:(k + 1) * 128], src_B[:, NBLK * c + k, :], id_sb)
            c0 = c * CH
            evac(dst_A[:, 1 + c0: 1 + c0 + CH], tps, eng)
            if c == 0:
                nc.gpsimd.tensor_copy(dst_A[:, L + 1: L + 2], dst_A[:, 1:2])
            if c == NCH - 1:
                nc.gpsimd.tensor_copy(dst_A[:, 0:1], dst_A[:, L:L + 1])

        def transpose_chunk_A2B(src_A, dst_B, c, eng):
            """PE-transpose layout-A columns into 4 layout-B blocks."""
            tps = ptr.tile([128, CH], f16, tag="tps", name="tpsb")
            c0 = c * CH
            for k in range(NBLK):
                nc.tensor.transpose(tps[:, k * 128:(k + 1) * 128],
                                    src_A[:, 1 + c0 + k * 128: 1 + c0 + (k + 1) * 128], id_sb)
            evac(dst_B[:, NBLK * c:NBLK * (c + 1), :].rearrange("p a b -> p (a b)"), tps, eng)

        for s in range(BS):
            # ---- projection: h0 = tokens @ Wp + bp (shared by both dirs) ----
            h0A = hA.tile([128, L + 2], f16, tag="hA_f", name="h0A")
            h0B = hB.tile([128, L // 128, 128], f16, tag="hB_f", name="h0B")
            for c in range(NCH):
                c0 = c * CH
                tokT = work.tile([8, CH], f16, tag="tokT", name="tokT")
                nc.sync.dma_start(out=tokT, in_=tokens_d[s].rearrange("l k -> k l")[:, c0:c0 + CH])
                pps = pmm.tile([128, CH], f32, tag="mps", name="pps")
                nc.tensor.matmul(pps, Wp_sb, tokT)
                nc.scalar.activation(h0A[:, 1 + c0: 1 + c0 + CH], pps,
                                     AF.Identity, bias=bpT_sb, scale=1.0)
                if c == 0:
                    nc.gpsimd.tensor_copy(h0A[:, L + 1: L + 2], h0A[:, 1:2])
                if c == NCH - 1:
                    nc.gpsimd.tensor_copy(h0A[:, 0:1], h0A[:, L:L + 1])
                transpose_chunk_A2B(h0A, h0B, c, "v")

            if debug_taps and s == 0:
                nc.sync.dma_start(out=dbg_h0_d[:].rearrange("(nb p) h -> p nb h", p=128), in_=h0B)

            cur_A = {0: h0A, 1: h0A}
            cur_B = {0: h0B, 1: h0B}
            for i in range(NL):
                last = i == NL - 1
                for d in (0, 1):
                    sfx = "fr"[d]
                    hA_in, hB_in = cur_A[d], cur_B[d]
                    hB_out = hB.tile([128, L // 128, 128], f16, tag=f"hB_{sfx}", name=f"hB{sfx}")
                    if not last:
                        hA_out = hA.tile([128, L + 2], f16, tag=f"hA_{sfx}", name=f"hAn{sfx}")
                    for c in range(NCH):
                        c0 = c * CH
                        mps = pmm.tile([128, CH], f32, name="mps")
                        for j in range(3):
                            off = c0 + (j if d == 0 else 2 - j)
                            nc.tensor.matmul(mps, W1_sb[:, i, j, :], hA_in[:, off:off + CH],
                                             start=(j == 0), stop=(j == 2))
                        # relu(mlp1 + b1): alternate DVE/ACT per chunk for balance
                        mlp1 = work.tile([128, CH], f16, tag="mlp1", name="mlp1")
                        if c % 2 == 0:
                            nc.vector.tensor_scalar(out=mlp1, in0=mps,
                                                    scalar1=b1T_sb[:, i:i + 1], scalar2=0.0,
                                                    op0=ALU.add, op1=ALU.max)
                        else:
                            nc.scalar.activation(mlp1, mps, AF.Relu,
                                                 bias=b1T_sb[:, i:i + 1], scale=1.0)
                        # y = h + mlp1 @ W2 accumulated in PSUM (identity-matmul residual)
                        yps = pyy.tile([128, CH], f32, name="yps")
                        for k in range(NBLK):
                            # one bank: only k==0 starts (PSUM zeroing is bank-granular)
                            nc.tensor.matmul(yps[:, k * 128:(k + 1) * 128],
                                             mlp1[:, k * 128:(k + 1) * 128], W2_sb[:, i, :],
                                             start=(k == 0), stop=False,
                                             skip_group_check=(k > 0))
                        nc.tensor.matmul(
                            yps, id_sb,
                            hB_in[:, NBLK * c:NBLK * (c + 1), :].rearrange("p a b -> p (a b)"),
                            start=False, stop=True)
                        if generic_b2:
                            for k in range(NBLK):
                                nc.vector.tensor_add(yps[:, k * 128:(k + 1) * 128],
                                                     yps[:, k * 128:(k + 1) * 128],
                                                     b2_sb[:, i, :])
                        stats = stat.tile([128, NBLK, 6], f32, tag="stats", name="stats")
                        mv = stat.tile([128, NBLK, 2], f32, tag="mv", name="mv")
                        for k in range(NBLK):
                            nc.vector.bn_stats(stats[:, k, :], yps[:, k * 128:(k + 1) * 128])
                            nc.vector.bn_aggr(mv[:, k, :], stats[:, k, :])
                        rstd = stat.tile([128, NBLK], f32, tag="rstd", name="rstd")
                        # last layer folds the 0.5 averaging: rsqrt(4(var+eps)) = 0.5*rstd
                        # (not valid in generic gamma/beta mode)
                        fold = last and not generic_gb
                        nc.scalar.activation(rstd, mv[:, :, 1], AF.Sqrt,
                                             bias=(eps4_sb if fold else eps_sb),
                                             scale=(4.0 if fold else 1.0))
                        nc.vector.reciprocal(rstd, rstd)
                        nmr = stat.tile([128, NBLK], f32, tag="nmr", name="nmr")
                        nc.vector.scalar_tensor_tensor(out=nmr, in0=mv[:, :, 0], scalar=-1.0,
                                                       in1=rstd, op0=ALU.mult, op1=ALU.mult)
                        e6_func = AF.Identity if generic_gb else AF.Relu
                        for k in range(NBLK):
                            nc.scalar.activation(hB_out[:, NBLK * c + k, :],
                                                 yps[:, k * 128:(k + 1) * 128], e6_func,
                                                 bias=nmr[:, k:k + 1], scale=rstd[:, k:k + 1])
                        if generic_gb:
                            for k in range(NBLK):
                                blk = hB_out[:, NBLK * c + k, :]
                                nc.vector.tensor_mul(blk, blk, gb_sb[:, 0, i, :])
                                nc.vector.tensor_add(blk, blk, gb_sb[:, 1, i, :])
                                nc.vector.tensor_scalar_max(out=blk, in0=blk, scalar1=0.0)
                        if not last:
                            transpose_chunk(hB_out, hA_out, c, "v" if c % 2 else "a")
                    if debug_taps and s == 0:
                        nc.sync.dma_start(
                            out=dbg_hB_d[i, d].rearrange("(nb p) h -> p nb h", p=128),
                            in_=hB_out)
                    cur_B[d] = hB_out
                    if not last:
                        cur_A[d] = hA_out

            # ---- finalize: h = hf + hr (0.5 pre-folded), head, softmax-pool ----
            havg = hB.tile([128, L // 128, 128], f32, tag="havg", name="havg")
            for c in range(NCH):
                sl = slice(NBLK * c, NBLK * (c + 1))
                nc.vector.tensor_add(havg[:, sl, :].rearrange("p a b -> p (a b)"),
                                     cur_B[0][:, sl, :].rearrange("p a b -> p (a b)"),
                                     cur_B[1][:, sl, :].rearrange("p a b -> p (a b)"))
                if generic_gb:
                    nc.vector.tensor_scalar_mul(out=havg[:, sl, :].rearrange("p a b -> p (a b)"),
                                                in0=havg[:, sl, :].rearrange("p a b -> p (a b)"),
                                                scalar1=0.5)
                nc.sync.dma_start(
                    out=h_out_d[s].rearrange("(nb p) h -> p nb h", p=128)[:, sl, :],
                    in_=havg[:, sl, :])
            havg_h = fin.tile([128, L // 128, 128], f16, tag="havg_h", bufs=1, name="havg_h")
            for c in range(NCH):
                sl = slice(NBLK * c, NBLK * (c + 1))
                nc.gpsimd.tensor_copy(havg_h[:, sl, :].rearrange("p a b -> p (a b)"),
                                      havg[:, sl, :].rearrange("p a b -> p (a b)"))
            scores = fin.tile([128, L // 128], f32, tag="scores", name="scores")
            scratch = fin.tile([128, H], f32, tag="scratch", name="scratch")
            for b in range(L // 128):
                nc.vector.scalar_tensor_tensor(out=scratch, in0=havg[:, b, :], scalar=1.0,
                                               in1=Wsbc_sb, op0=ALU.mult, op1=ALU.mult,
                                               accum_out=scores[:, b:b + 1])
            esc = fin.tile([128, L // 128], f16, tag="esc", name="esc")
            nc.scalar.activation(esc, scores, AF.Exp)
            ered = fin.tile([128, 1], f32, tag="ered", name="ered")
            nc.vector.reduce_sum(ered, esc, axis=mybir.AxisListType.X)
            ups = pus.tile([1, 129], f32, name="ups")
            for b in range(L // 128):
                nc.tensor.matmul(ups[:, 0:128], esc[:, b:b + 1], havg_h[:, b, :],
                                 start=(b == 0), stop=(b == L // 128 - 1))
            nc.tensor.matmul(ups[:, 128:129], ered, ones_sb)
            rS = fin.tile([1, 1], f32, tag="rS", name="rS")
            nc.vector.reciprocal(rS, ups[:, 128:129])
            pooled_sb = fin.tile([1, H], f32, tag="pooled_sb", name="pooled_sb")
            nc.vector.tensor_scalar_mul(out=pooled_sb, in0=ups[:, 0:128], scalar1=rS)
            nc.sync.dma_start(out=pooled_d[s:s + 1, :], in_=pooled_sb)

        for p in (pus, ptr, pyy, pmm, fin, stat, work, hB, hA, con):
            p.release()

    nc.compile()
    return nc


def kernel(tokens, Wp, bp, W1, b1, W2, b2, gamma, beta, Ws, bs):
    tokens = np.ascontiguousarray(np.asarray(tokens, np.float32))
    B, L, K = tokens.shape
    BS = B // CORES

    generic_b2 = not np.all(np.asarray(b2) == 0)
    generic_gb = not (np.all(np.asarray(gamma) == 1) and np.all(np.asarray(beta) == 0))

    nc = _build(BS, L, generic_b2=generic_b2, generic_gb=generic_gb)

    base = {
        "Wp": np.ascontiguousarray(np.asarray(Wp).astype(np.float16)),
        "W1h": np.ascontiguousarray(np.asarray(W1).astype(np.float16)),
        "W2h": np.ascontiguousarray(np.asarray(W2).astype(np.float16)),
        "b1T": np.ascontiguousarray(np.asarray(b1, np.float32).T.reshape(H, NL)),
        "bpT": np.ascontiguousarray(np.asarray(bp, np.float32).reshape(H, 1)),
        "Ws_bc": np.ascontiguousarray(np.broadcast_to(np.asarray(Ws, np.float32).reshape(1, H), (H, H))),
        "id128h": np.eye(H, dtype=np.float16),
        "ones128": np.ones((H, 1), np.float32),
    }
    if generic_b2:
        base["b2bc"] = np.ascontiguousarray(
            np.broadcast_to(np.asarray(b2, np.float32).reshape(1, NL * H), (H, NL * H)))
    if generic_gb:
        gb = np.stack([np.asarray(gamma, np.float32), np.asarray(beta, np.float32)])
        base["gb_bc"] = np.ascontiguousarray(
            np.broadcast_to(gb.reshape(1, 2 * NL * H), (H, 2 * NL * H)))

    tokens_h = tokens.astype(np.float16)
    in_maps = [dict(base, tokens_sh=np.ascontiguousarray(tokens_h[i * BS:(i + 1) * BS]))
               for i in range(CORES)]

    from concourse.bass_utils import run_bass_kernel_spmd
    trace = bool(os.environ.get("BCE_TRACE"))
    res = run_bass_kernel_spmd(nc, in_maps, core_ids=list(range(CORES)), trace=trace)
    globals()["_last_results"] = res

    h = np.concatenate([r["h_out"] for r in res.results], axis=0)
    pooled = np.concatenate([r["pooled_out"] for r in res.results], axis=0)
    return (h, pooled)


# revision 37
# speedup vs baseline: 1.0061x; 1.0061x over previous
"""Trainium2 Bass kernel for BoundaryCycleEncoder.

Architecture notes:
- Pure data parallel: 64 samples sharded 8 per core across 8 NeuronCores.
- The reverse direction encode(flip(x)) flipped back equals running the same
  encoder with the prev/next blocks of W1 swapped, so no data flipping is done;
  both directions share the projected h0.
- Per layer, h is kept in two layouts (all matmul-path data in fp16):
    A: [H=128 partitions, L tokens] (with +-1 halo columns) feeds the matmuls
    B: [128 tokens, H] is where LayerNorm stats/normalization happen
  mlp1 = relu(W1 . [prev;self;next]) is computed weight-stationary into PSUM
  (layout A); mlp2 runs activation-stationary (lhsT = mlp1 128-token chunks)
  landing delta in layout B PSUM, where one identity-matmul accumulates the
  residual h so y = h + delta materializes directly in PSUM; bn_stats reads
  PSUM per 128-token block; normalize+relu is one ScalarE activation per block
  with per-partition (per-token) scale/bias; PE transposes bring h back to
  layout A. PSUM->SBUF evacuations alternate DVE/ACT to balance engines.
"""

import os
os.environ.setdefault("NEURON_RT_RESET_CORES", "1")  # recover from wedged cores

import numpy as np
import ml_dtypes

H = 128
NL = 4
EPS = 1e-5
CORES = 8
CH = 512           # tokens per chunk (1 PSUM bank at fp32)
NBLK = CH // 128   # 128-token blocks per chunk


def _build(BS, L, generic_b2=False, generic_gb=False, debug_taps=False):
    """Build the SPMD Bass program for one core holding BS samples of length L."""
    import concourse.bacc as bacc
    import concourse.mybir as mybir
    import concourse.tile as tile

    NCH = L // CH
    f32 = mybir.dt.float32
    f16 = mybir.dt.float16
    AF = mybir.ActivationFunctionType
    ALU = mybir.AluOpType

    nc = bacc.Bacc()

    tokens_d = nc.dram_tensor("tokens_sh", [BS, L, 8], f16, kind="ExternalInput")
    Wp_d = nc.dram_tensor("Wp", [8, H], f16, kind="ExternalInput")
    W1_d = nc.dram_tensor("W1h", [NL, 3 * H, H], f16, kind="ExternalInput")
    W2_d = nc.dram_tensor("W2h", [NL, H, H], f16, kind="ExternalInput")
    b1T_d = nc.dram_tensor("b1T", [H, NL], f32, kind="ExternalInput")
    bpT_d = nc.dram_tensor("bpT", [H, 1], f32, kind="ExternalInput")
    Wsbc_d = nc.dram_tensor("Ws_bc", [H, H], f32, kind="ExternalInput")
    id_d = nc.dram_tensor("id128h", [H, H], f16, kind="ExternalInput")
    ones_d = nc.dram_tensor("ones128", [H, 1], f32, kind="ExternalInput")
    if generic_b2:
        b2_d = nc.dram_tensor("b2bc", [H, NL * H], f32, kind="ExternalInput")
    if generic_gb:
        gb_d = nc.dram_tensor("gb_bc", [H, 2 * NL * H], f32, kind="ExternalInput")

    h_out_d = nc.dram_tensor("h_out", [BS, L, H], f32, kind="ExternalOutput")
    pooled_d = nc.dram_tensor("pooled_out", [BS, H], f32, kind="ExternalOutput")
    if debug_taps:
        dbg_h0_d = nc.dram_tensor("dbg_h0", [L, H], f16, kind="ExternalOutput")
        dbg_hB_d = nc.dram_tensor("dbg_hB", [NL, 2, L, H], f16, kind="ExternalOutput")

    with tile.TileContext(nc) as tc:
        con = tc.alloc_tile_pool(name="con", bufs=1)      # constants/weights
        hA = tc.alloc_tile_pool(name="hA", bufs=2)        # [128, L+2] layout-A h (f16)
        hB = tc.alloc_tile_pool(name="hB", bufs=2)        # [128, L/128, 128] layout-B h (f16)
        work = tc.alloc_tile_pool(name="work", bufs=4)    # per-chunk tiles
        stat = tc.alloc_tile_pool(name="stat", bufs=6)    # small stats tiles
        fin = tc.alloc_tile_pool(name="fin", bufs=2)      # final-stage tiles
        pmm = tc.alloc_tile_pool(name="pmm", bufs=2, space="PSUM")
        pyy = tc.alloc_tile_pool(name="pyy", bufs=3, space="PSUM")
        ptr = tc.alloc_tile_pool(name="ptr", bufs=2, space="PSUM")
        pus = tc.alloc_tile_pool(name="pus", bufs=1, space="PSUM")

        # ---- load constants ----
        W1_sb = con.tile([128, NL, 3, H], f16)
        nc.sync.dma_start(out=W1_sb, in_=W1_d[:].rearrange("nl (j p) h -> p nl j h", p=128))
        W2_sb = con.tile([128, NL, H], f16)
        nc.sync.dma_start(out=W2_sb, in_=W2_d[:].rearrange("nl p h -> p nl h"))
        Wp_sb = con.tile([8, H], f16)
        nc.sync.dma_start(out=Wp_sb, in_=Wp_d[:])
        b1T_sb = con.tile([128, NL], f32)
        nc.sync.dma_start(out=b1T_sb, in_=b1T_d[:])
        bpT_sb = con.tile([128, 1], f32)
        nc.sync.dma_start(out=bpT_sb, in_=bpT_d[:])
        Wsbc_sb = con.tile([128, H], f32)
        nc.sync.dma_start(out=Wsbc_sb, in_=Wsbc_d[:])
        id_sb = con.tile([128, H], f16)
        nc.sync.dma_start(out=id_sb, in_=id_d[:])
        ones_sb = con.tile([128, 1], f32)
        nc.sync.dma_start(out=ones_sb, in_=ones_d[:])
        if generic_b2:
            b2_sb = con.tile([128, NL, H], f32)
            nc.sync.dma_start(out=b2_sb, in_=b2_d[:].rearrange("p (nl h) -> p nl h", nl=NL))
        if generic_gb:
            gb_sb = con.tile([128, 2, NL, H], f32)
            nc.sync.dma_start(out=gb_sb, in_=gb_d[:].rearrange("p (t nl h) -> p t nl h", t=2, nl=NL))
        eps_sb = con.tile([128, 1], f32)
        nc.vector.memset(eps_sb, EPS)
        eps4_sb = con.tile([128, 1], f32)
        nc.vector.memset(eps4_sb, 4.0 * EPS)

        def evac(dst, src, eng):
            """PSUM->SBUF copy on the chosen engine ('v' DVE / 'a' ACT)."""
            if eng == "v":
                nc.vector.tensor_copy(dst, src)
            else:
                nc.scalar.activation(dst, src, AF.Copy)

        def transpose_chunk(src_B, dst_A, c, eng):
            """PE-transpose 4 blocks of layout-B h into layout-A columns (+halo)."""
            tps = ptr.tile([128, CH], f16, tag="tps", name="tps")
            for k in range(NBLK):
                nc.tensor.transpose(tps[:, k * 128:(k + 1) * 128], src_B[:, NBLK * c + k, :], id_sb)
            c0 = c * CH
            evac(dst_A[:, 1 + c0: 1 + c0 + CH], tps, eng)
            if c == 0:
                nc.gpsimd.tensor_copy(dst_A[:, L + 1: L + 2], dst_A[:, 1:2])
            if c == NCH - 1:
                nc.gpsimd.tensor_copy(dst_A[:, 0:1], dst_A[:, L:L + 1])

        def transpose_chunk_A2B(src_A, dst_B, c, eng):
            """PE-transpose layout-A columns into 4 layout-B blocks."""
            tps = ptr.tile([128, CH], f16, tag="tps", name="tpsb")
            c0 = c * CH
            for k in range(NBLK):
                nc.tensor.transpose(tps[:, k * 128:(k + 1) * 128],
                                    src_A[:, 1 + c0 + k * 128: 1 + c0 + (k + 1) * 128], id_sb)
            evac(dst_B[:, NBLK * c:NBLK * (c + 1), :].rearrange("p a b -> p (a b)"), tps, eng)

        for s in range(BS):
            # ---- projection: h0 = tokens @ Wp + bp (shared by both dirs) ----
            h0A = hA.tile([128, L + 2], f16, tag="hA_f", name="h0A")
            h0B = hB.tile([128, L // 128, 128], f16, tag="hB_f", name="h0B")
            for c in range(NCH):
                c0 = c * CH
                tokT = work.tile([8, CH], f16, tag="tokT", name="tokT")
                nc.sync.dma_start(out=tokT, in_=tokens_d[s].rearrange("l k -> k l")[:, c0:c0 + CH])
                pps = pmm.tile([128, CH], f32, tag="mps", name="pps")
                nc.tensor.matmul(pps, Wp_sb, tokT)
                nc.scalar.activation(h0A[:, 1 + c0: 1 + c0 + CH], pps,
                                     AF.Identity, bias=bpT_sb, scale=1.0)
                if c == 0:
                    nc.gpsimd.tensor_copy(h0A[:, L + 1: L + 2], h0A[:, 1:2])
                if c == NCH - 1:
                    nc.gpsimd.tensor_copy(h0A[:, 0:1], h0A[:, L:L + 1])
                transpose_chunk_A2B(h0A, h0B, c, "v")

            if debug_taps and s == 0:
                nc.sync.dma_start(out=dbg_h0_d[:].rearrange("(nb p) h -> p nb h", p=128), in_=h0B)

            cur_A = {0: h0A, 1: h0A}
            cur_B = {0: h0B, 1: h0B}
            for i in range(NL):
                last = i == NL - 1
                for d in (0, 1):
                    sfx = "fr"[d]
                    hA_in, hB_in = cur_A[d], cur_B[d]
                    hB_out = hB.tile([128, L // 128, 128], f16, tag=f"hB_{sfx}", name=f"hB{sfx}")
                    if not last:
                        hA_out = hA.tile([128, L + 2], f16, tag=f"hA_{sfx}", name=f"hAn{sfx}")
                    for c in range(NCH):
                        c0 = c * CH
                        mps = pmm.tile([128, CH], f32, name="mps")
                        for j in range(3):
                            off = c0 + (j if d == 0 else 2 - j)
                            nc.tensor.matmul(mps, W1_sb[:, i, j, :], hA_in[:, off:off + CH],
                                             start=(j == 0), stop=(j == 2))
                        # relu(mlp1 + b1): alternate DVE/ACT per chunk for balance
                        mlp1 = work.tile([128, CH], f16, tag="mlp1", name="mlp1")
                        nc.scalar.activation(mlp1, mps, AF.Relu,
                                             bias=b1T_sb[:, i:i + 1], scale=1.0)
                        # y = h + mlp1 @ W2 accumulated in PSUM (identity-matmul residual)
                        yps = pyy.tile([128, CH], f32, name="yps")
                        for k in range(NBLK):
                            # one bank: only k==0 starts (PSUM zeroing is bank-granular)
                            nc.tensor.matmul(yps[:, k * 128:(k + 1) * 128],
                                             mlp1[:, k * 128:(k + 1) * 128], W2_sb[:, i, :],
                                             start=(k == 0), stop=False,
                                             skip_group_check=(k > 0))
                        nc.tensor.matmul(
                            yps, id_sb,
                            hB_in[:, NBLK * c:NBLK * (c + 1), :].rearrange("p a b -> p (a b)"),
                            start=False, stop=True)
                        if generic_b2:
                            for k in range(NBLK):
                                nc.vector.tensor_add(yps[:, k * 128:(k + 1) * 128],
                                                     yps[:, k * 128:(k + 1) * 128],
                                                     b2_sb[:, i, :])
                        stats = stat.tile([128, NBLK, 6], f32, tag="stats", name="stats")
                        mv = stat.tile([128, NBLK, 2], f32, tag="mv", name="mv")
                        for k in range(NBLK):
                            nc.vector.bn_stats(stats[:, k, :], yps[:, k * 128:(k + 1) * 128])
                            nc.vector.bn_aggr(mv[:, k, :], stats[:, k, :])
                        rstd = stat.tile([128, NBLK], f32, tag="rstd", name="rstd")
                        # last layer folds the 0.5 averaging: rsqrt(4(var+eps)) = 0.5*rstd
                        # (not valid in generic gamma/beta mode)
                        fold = last and not generic_gb
                        nc.scalar.activation(rstd, mv[:, :, 1], AF.Sqrt,
                                             bias=(eps4_sb if fold else eps_sb),
                                             scale=(4.0 if fold else 1.0))
                        nc.vector.reciprocal(rstd, rstd)
                        nmr = stat.tile([128, NBLK], f32, tag="nmr", name="nmr")
                        nc.vector.scalar_tensor_tensor(out=nmr, in0=mv[:, :, 0], scalar=-1.0,
                                                       in1=rstd, op0=ALU.mult, op1=ALU.mult)
                        e6_func = AF.Identity if generic_gb else AF.Relu
                        for k in range(NBLK):
                            nc.scalar.activation(hB_out[:, NBLK * c + k, :],
                                                 yps[:, k * 128:(k + 1) * 128], e6_func,
                                                 bias=nmr[:, k:k + 1], scale=rstd[:, k:k + 1])
                        if generic_gb:
                            for k in range(NBLK):
                                blk = hB_out[:, NBLK * c + k, :]
                                nc.vector.tensor_mul(blk, blk, gb_sb[:, 0, i, :])
                                nc.vector.tensor_add(blk, blk, gb_sb[:, 1, i, :])
                                nc.vector.tensor_scalar_max(out=blk, in0=blk, scalar1=0.0)
                        if not last:
                            transpose_chunk(hB_out, hA_out, c, "v" if c % 2 else "a")
                    if debug_taps and s == 0:
                        nc.sync.dma_start(
                            out=dbg_hB_d[i, d].rearrange("(nb p) h -> p nb h", p=128),
                            in_=hB_out)
                    cur_B[d] = hB_out
                    if not last:
                        cur_A[d] = hA_out

            # ---- finalize: h = hf + hr (0.5 pre-folded), head, softmax-pool ----
            havg = hB.tile([128, L // 128, 128], f32, tag="havg", name="havg")
            for c in range(NCH):
                sl = slice(NBLK * c, NBLK * (c + 1))
                nc.vector.tensor_add(havg[:, sl, :].rearrange("p a b -> p (a b)"),
                                     cur_B[0][:, sl, :].rearrange("p a b -> p (a b)"),
                                     cur_B[1][:, sl, :].rearrange("p a b -> p (a b)"))
                if generic_gb:
                    nc.vector.tensor_scalar_mul(out=havg[:, sl, :].rearrange("p a b -> p (a b)"),
                                                in0=havg[:, sl, :].rearrange("p a b -> p (a b)"),
                                                scalar1=0.5)
                nc.sync.dma_start(
                    out=h_out_d[s].rearrange("(nb p) h -> p nb h", p=128)[:, sl, :],
                    in_=havg[:, sl, :])
            havg_h = fin.tile([128, L // 128, 128], f16, tag="havg_h", bufs=1, name="havg_h")
            for c in range(NCH):
                sl = slice(NBLK * c, NBLK * (c + 1))
                nc.gpsimd.tensor_copy(havg_h[:, sl, :].rearrange("p a b -> p (a b)"),
                                      havg[:, sl, :].rearrange("p a b -> p (a b)"))
            scores = fin.tile([128, L // 128], f32, tag="scores", name="scores")
            scratch = fin.tile([128, H], f32, tag="scratch", name="scratch")
            for b in range(L // 128):
                nc.vector.scalar_tensor_tensor(out=scratch, in0=havg[:, b, :], scalar=1.0,
                                               in1=Wsbc_sb, op0=ALU.mult, op1=ALU.mult,
                                               accum_out=scores[:, b:b + 1])
            esc = fin.tile([128, L // 128], f16, tag="esc", name="esc")
            nc.scalar.activation(esc, scores, AF.Exp)
            ered = fin.tile([128, 1], f32, tag="ered", name="ered")
            nc.vector.reduce_sum(ered, esc, axis=mybir.AxisListType.X)
            ups = pus.tile([1, 129], f32, name="ups")
            for b in range(L // 128):
                nc.tensor.matmul(ups[:, 0:128], esc[:, b:b + 1], havg_h[:, b, :],
                                 start=(b == 0), stop=(b == L // 128 - 1))
            nc.tensor.matmul(ups[:, 128:129], ered, ones_sb)
            rS = fin.tile([1, 1], f32, tag="rS", name="rS")
            nc.vector.reciprocal(rS, ups[:, 128:129])
            pooled_sb = fin.tile([1, H], f32, tag="pooled_sb", name="pooled_sb")
            nc.vector.tensor_scalar_mul(out=pooled_sb, in0=ups[:, 0:128], scalar1=rS)
            nc.sync.dma_start(out=pooled_d[s:s + 1, :], in_=pooled_sb)

        for p in (pus, ptr, pyy, pmm, fin, stat, work, hB, hA, con):
            p.release()

    nc.compile()
    return nc


def kernel(tokens, Wp, bp, W1, b1, W2, b2, gamma, beta, Ws, bs):
    tokens = np.ascontiguousarray(np.asarray(tokens, np.float32))
    B, L, K = tokens.shape
    BS = B // CORES

    generic_b2 = not np.all(np.asarray(b2) == 0)
    generic_gb = not (np.all(np.asarray(gamma) == 1) and np.all(np.asarray(beta) == 0))

    nc = _build(BS, L, generic_b2=generic_b2, generic_gb=generic_gb)

    base = {
        "Wp": np.ascontiguousarray(np.asarray(Wp).astype(np.float16)),
        "W1h": np.ascontiguousarray(np.asarray(W1).astype(np.float16)),
        "W2h": np.ascontiguousarray(np.asarray(W2).astype(np.float16)),
        "b1T": np.ascontiguousarray(np.asarray(b1, np.float32).T.reshape(H, NL)),
        "bpT": np.ascontiguousarray(np.asarray(bp, np.float32).reshape(H, 1)),
        "Ws_bc": np.ascontiguousarray(np.broadcast_to(np.asarray(Ws, np.float32).reshape(1, H), (H, H))),
        "id128h": np.eye(H, dtype=np.float16),
        "ones128": np.ones((H, 1), np.float32),
    }
    if generic_b2:
        base["b2bc"] = np.ascontiguousarray(
            np.broadcast_to(np.asarray(b2, np.float32).reshape(1, NL * H), (H, NL * H)))
    if generic_gb:
        gb = np.stack([np.asarray(gamma, np.float32), np.asarray(beta, np.float32)])
        base["gb_bc"] = np.ascontiguousarray(
            np.broadcast_to(gb.reshape(1, 2 * NL * H), (H, 2 * NL * H)))

    tokens_h = tokens.astype(np.float16)
    in_maps = [dict(base, tokens_sh=np.ascontiguousarray(tokens_h[i * BS:(i + 1) * BS]))
               for i in range(CORES)]

    from concourse.bass_utils import run_bass_kernel_spmd
    trace = bool(os.environ.get("BCE_TRACE"))
    res = run_bass_kernel_spmd(nc, in_maps, core_ids=list(range(CORES)), trace=trace)
    globals()["_last_results"] = res

    h = np.concatenate([r["h_out"] for r in res.results], axis=0)
    pooled = np.concatenate([r["pooled_out"] for r in res.results], axis=0)
    return (h, pooled)


# revision 38
# speedup vs baseline: 1.0205x; 1.0143x over previous
"""Trainium2 Bass kernel for BoundaryCycleEncoder.

Architecture notes:
- Pure data parallel: 64 samples sharded 8 per core across 8 NeuronCores.
- The reverse direction encode(flip(x)) flipped back equals running the same
  encoder with the prev/next blocks of W1 swapped, so no data flipping is done;
  both directions share the projected h0.
- Per layer, h is kept in two layouts (all matmul-path data in fp16):
    A: [H=128 partitions, L tokens] (with +-1 halo columns) feeds the matmuls
    B: [128 tokens, H] is where LayerNorm stats/normalization happen
  mlp1 = relu(W1 . [prev;self;next]) is computed weight-stationary into PSUM
  (layout A); mlp2 runs activation-stationary (lhsT = mlp1 128-token chunks)
  landing delta in layout B PSUM, where one identity-matmul accumulates the
  residual h so y = h + delta materializes directly in PSUM; bn_stats reads
  PSUM per 128-token block; normalize+relu is one ScalarE activation per block
  with per-partition (per-token) scale/bias; PE transposes bring h back to
  layout A. PSUM->SBUF evacuations alternate DVE/ACT to balance engines.
"""

import os
os.environ.setdefault("NEURON_RT_RESET_CORES", "1")  # recover from wedged cores

import numpy as np
import ml_dtypes

H = 128
NL = 4
EPS = 1e-5
CORES = 8
CH = 512           # tokens per chunk (1 PSUM bank at fp32)
NBLK = CH // 128   # 128-token blocks per chunk


def _build(BS, L, generic_b2=False, generic_gb=False, debug_taps=False):
    """Build the SPMD Bass program for one core holding BS samples of length L."""
    import concourse.bacc as bacc
    import concourse.mybir as mybir
    import concourse.tile as tile

    NCH = L // CH
    f32 = mybir.dt.float32
    f16 = mybir.dt.float16
    AF = mybir.ActivationFunctionType
    ALU = mybir.AluOpType

    nc = bacc.Bacc()

    tokens_d = nc.dram_tensor("tokens_sh", [BS, L, 8], f16, kind="ExternalInput")
    Wp_d = nc.dram_tensor("Wp", [8, H], f16, kind="ExternalInput")
    W1_d = nc.dram_tensor("W1h", [NL, 3 * H, H], f16, kind="ExternalInput")
    W2_d = nc.dram_tensor("W2h", [NL, H, H], f16, kind="ExternalInput")
    b1T_d = nc.dram_tensor("b1T", [H, NL], f32, kind="ExternalInput")
    bpT_d = nc.dram_tensor("bpT", [H, 1], f32, kind="ExternalInput")
    Wsbc_d = nc.dram_tensor("Ws_bc", [H, H], f32, kind="ExternalInput")
    id_d = nc.dram_tensor("id128h", [H, H], f16, kind="ExternalInput")
    ones_d = nc.dram_tensor("ones128", [H, 1], f32, kind="ExternalInput")
    if generic_b2:
        b2_d = nc.dram_tensor("b2bc", [H, NL * H], f32, kind="ExternalInput")
    if generic_gb:
        gb_d = nc.dram_tensor("gb_bc", [H, 2 * NL * H], f32, kind="ExternalInput")

    h_out_d = nc.dram_tensor("h_out", [BS, L, H], f32, kind="ExternalOutput")
    pooled_d = nc.dram_tensor("pooled_out", [BS, H], f32, kind="ExternalOutput")
    if debug_taps:
        dbg_h0_d = nc.dram_tensor("dbg_h0", [L, H], f16, kind="ExternalOutput")
        dbg_hB_d = nc.dram_tensor("dbg_hB", [NL, 2, L, H], f16, kind="ExternalOutput")

    with tile.TileContext(nc) as tc:
        con = tc.alloc_tile_pool(name="con", bufs=1)      # constants/weights
        hA = tc.alloc_tile_pool(name="hA", bufs=3)        # [128, L+2] layout-A h (f16)
        hB = tc.alloc_tile_pool(name="hB", bufs=3)        # [128, L/128, 128] layout-B h (f16)
        work = tc.alloc_tile_pool(name="work", bufs=6)    # per-chunk tiles
        stat = tc.alloc_tile_pool(name="stat", bufs=6)    # small stats tiles
        fin = tc.alloc_tile_pool(name="fin", bufs=2)      # final-stage tiles
        pmm = tc.alloc_tile_pool(name="pmm", bufs=2, space="PSUM")
        pyy = tc.alloc_tile_pool(name="pyy", bufs=3, space="PSUM")
        ptr = tc.alloc_tile_pool(name="ptr", bufs=2, space="PSUM")
        pus = tc.alloc_tile_pool(name="pus", bufs=1, space="PSUM")

        # ---- load constants ----
        W1_sb = con.tile([128, NL, 3, H], f16)
        nc.sync.dma_start(out=W1_sb, in_=W1_d[:].rearrange("nl (j p) h -> p nl j h", p=128))
        W2_sb = con.tile([128, NL, H], f16)
        nc.sync.dma_start(out=W2_sb, in_=W2_d[:].rearrange("nl p h -> p nl h"))
        Wp_sb = con.tile([8, H], f16)
        nc.sync.dma_start(out=Wp_sb, in_=Wp_d[:])
        b1T_sb = con.tile([128, NL], f32)
        nc.sync.dma_start(out=b1T_sb, in_=b1T_d[:])
        bpT_sb = con.tile([128, 1], f32)
        nc.sync.dma_start(out=bpT_sb, in_=bpT_d[:])
        Wsbc_sb = con.tile([128, H], f32)
        nc.sync.dma_start(out=Wsbc_sb, in_=Wsbc_d[:])
        id_sb = con.tile([128, H], f16)
        nc.sync.dma_start(out=id_sb, in_=id_d[:])
        ones_sb = con.tile([128, 1], f32)
        nc.sync.dma_start(out=ones_sb, in_=ones_d[:])
        if generic_b2:
            b2_sb = con.tile([128, NL, H], f32)
            nc.sync.dma_start(out=b2_sb, in_=b2_d[:].rearrange("p (nl h) -> p nl h", nl=NL))
        if generic_gb:
            gb_sb = con.tile([128, 2, NL, H], f32)
            nc.sync.dma_start(out=gb_sb, in_=gb_d[:].rearrange("p (t nl h) -> p t nl h", t=2, nl=NL))
        eps_sb = con.tile([128, 1], f32)
        nc.vector.memset(eps_sb, EPS)
        eps4_sb = con.tile([128, 1], f32)
        nc.vector.memset(eps4_sb, 4.0 * EPS)

        def evac(dst, src, eng):
            """PSUM->SBUF copy on the chosen engine ('v' DVE / 'a' ACT)."""
            if eng == "v":
                nc.vector.tensor_copy(dst, src)
            else:
                nc.scalar.activation(dst, src, AF.Copy)

        def transpose_chunk(src_B, dst_A, c, eng):
            """PE-transpose 4 blocks of layout-B h into layout-A columns (+halo)."""
            tps = ptr.tile([128, CH], f16, tag="tps", name="tps")
            for k in range(NBLK):
                nc.tensor.transpose(tps[:, k * 128:(k + 1) * 128], src_B[:, NBLK * c + k, :], id_sb)
            c0 = c * CH
            evac(dst_A[:, 1 + c0: 1 + c0 + CH], tps, eng)
            if c == 0:
                nc.gpsimd.tensor_copy(dst_A[:, L + 1: L + 2], dst_A[:, 1:2])
            if c == NCH - 1:
                nc.gpsimd.tensor_copy(dst_A[:, 0:1], dst_A[:, L:L + 1])

        def transpose_chunk_A2B(src_A, dst_B, c, eng):
            """PE-transpose layout-A columns into 4 layout-B blocks."""
            tps = ptr.tile([128, CH], f16, tag="tps", name="tpsb")
            c0 = c * CH
            for k in range(NBLK):
                nc.tensor.transpose(tps[:, k * 128:(k + 1) * 128],
                                    src_A[:, 1 + c0 + k * 128: 1 + c0 + (k + 1) * 128], id_sb)
            evac(dst_B[:, NBLK * c:NBLK * (c + 1), :].rearrange("p a b -> p (a b)"), tps, eng)

        for s in range(BS):
            # ---- projection: h0 = tokens @ Wp + bp (shared by both dirs) ----
            h0A = hA.tile([128, L + 2], f16, tag="hA_f", name="h0A")
            h0B = hB.tile([128, L // 128, 128], f16, tag="hB_f", name="h0B")
            for c in range(NCH):
                c0 = c * CH
                tokT = work.tile([8, CH], f16, tag="tokT", name="tokT")
                nc.sync.dma_start(out=tokT, in_=tokens_d[s].rearrange("l k -> k l")[:, c0:c0 + CH])
                pps = pmm.tile([128, CH], f32, tag="mps", name="pps")
                nc.tensor.matmul(pps, Wp_sb, tokT)
                nc.scalar.activation(h0A[:, 1 + c0: 1 + c0 + CH], pps,
                                     AF.Identity, bias=bpT_sb, scale=1.0)
                if c == 0:
                    nc.gpsimd.tensor_copy(h0A[:, L + 1: L + 2], h0A[:, 1:2])
                if c == NCH - 1:
                    nc.gpsimd.tensor_copy(h0A[:, 0:1], h0A[:, L:L + 1])
                transpose_chunk_A2B(h0A, h0B, c, "v")

            if debug_taps and s == 0:
                nc.sync.dma_start(out=dbg_h0_d[:].rearrange("(nb p) h -> p nb h", p=128), in_=h0B)

            cur_A = {0: h0A, 1: h0A}
            cur_B = {0: h0B, 1: h0B}
            for i in range(NL):
                last = i == NL - 1
                for d in (0, 1):
                    sfx = "fr"[d]
                    hA_in, hB_in = cur_A[d], cur_B[d]
                    hB_out = hB.tile([128, L // 128, 128], f16, tag=f"hB_{sfx}", name=f"hB{sfx}")
                    if not last:
                        hA_out = hA.tile([128, L + 2], f16, tag=f"hA_{sfx}", name=f"hAn{sfx}")
                    for c in range(NCH):
                        c0 = c * CH
                        mps = pmm.tile([128, CH], f32, name="mps")
                        for j in range(3):
                            off = c0 + (j if d == 0 else 2 - j)
                            nc.tensor.matmul(mps, W1_sb[:, i, j, :], hA_in[:, off:off + CH],
                                             start=(j == 0), stop=(j == 2))
                        # relu(mlp1 + b1): alternate DVE/ACT per chunk for balance
                        mlp1 = work.tile([128, CH], f16, tag="mlp1", name="mlp1")
                        nc.scalar.activation(mlp1, mps, AF.Relu,
                                             bias=b1T_sb[:, i:i + 1], scale=1.0)
                        # y = h + mlp1 @ W2 accumulated in PSUM (identity-matmul residual)
                        yps = pyy.tile([128, CH], f32, name="yps")
                        for k in range(NBLK):
                            # one bank: only k==0 starts (PSUM zeroing is bank-granular)
                            nc.tensor.matmul(yps[:, k * 128:(k + 1) * 128],
                                             mlp1[:, k * 128:(k + 1) * 128], W2_sb[:, i, :],
                                             start=(k == 0), stop=False,
                                             skip_group_check=(k > 0))
                        nc.tensor.matmul(
                            yps, id_sb,
                            hB_in[:, NBLK * c:NBLK * (c + 1), :].rearrange("p a b -> p (a b)"),
                            start=False, stop=True)
                        if generic_b2:
                            for k in range(NBLK):
                                nc.vector.tensor_add(yps[:, k * 128:(k + 1) * 128],
                                                     yps[:, k * 128:(k + 1) * 128],
                                                     b2_sb[:, i, :])
                        stats = stat.tile([128, NBLK, 6], f32, tag="stats", name="stats")
                        mv = stat.tile([128, NBLK, 2], f32, tag="mv", name="mv")
                        for k in range(NBLK):
                            nc.vector.bn_stats(stats[:, k, :], yps[:, k * 128:(k + 1) * 128])
                            nc.vector.bn_aggr(mv[:, k, :], stats[:, k, :])
                        rstd = stat.tile([128, NBLK], f32, tag="rstd", name="rstd")
                        # last layer folds the 0.5 averaging: rsqrt(4(var+eps)) = 0.5*rstd
                        # (not valid in generic gamma/beta mode)
                        fold = last and not generic_gb
                        nc.scalar.activation(rstd, mv[:, :, 1], AF.Sqrt,
                                             bias=(eps4_sb if fold else eps_sb),
                                             scale=(4.0 if fold else 1.0))
                        nc.vector.reciprocal(rstd, rstd)
                        nmr = stat.tile([128, NBLK], f32, tag="nmr", name="nmr")
                        nc.vector.scalar_tensor_tensor(out=nmr, in0=mv[:, :, 0], scalar=-1.0,
                                                       in1=rstd, op0=ALU.mult, op1=ALU.mult)
                        e6_func = AF.Identity if generic_gb else AF.Relu
                        for k in range(NBLK):
                            nc.scalar.activation(hB_out[:, NBLK * c + k, :],
                                                 yps[:, k * 128:(k + 1) * 128], e6_func,
                                                 bias=nmr[:, k:k + 1], scale=rstd[:, k:k + 1])
                        if generic_gb:
                            for k in range(NBLK):
                                blk = hB_out[:, NBLK * c + k, :]
                                nc.vector.tensor_mul(blk, blk, gb_sb[:, 0, i, :])
                                nc.vector.tensor_add(blk, blk, gb_sb[:, 1, i, :])
                                nc.vector.tensor_scalar_max(out=blk, in0=blk, scalar1=0.0)
                        if not last:
                            transpose_chunk(hB_out, hA_out, c, "v" if c % 2 else "a")
                    if debug_taps and s == 0:
                        nc.sync.dma_start(
                            out=dbg_hB_d[i, d].rearrange("(nb p) h -> p nb h", p=128),
                            in_=hB_out)
                    cur_B[d] = hB_out
                    if not last:
                        cur_A[d] = hA_out

            # ---- finalize: h = hf + hr (0.5 pre-folded), head, softmax-pool ----
            havg = hB.tile([128, L // 128, 128], f32, tag="havg", name="havg")
            for c in range(NCH):
                sl = slice(NBLK * c, NBLK * (c + 1))
                nc.vector.tensor_add(havg[:, sl, :].rearrange("p a b -> p (a b)"),
                                     cur_B[0][:, sl, :].rearrange("p a b -> p (a b)"),
                                     cur_B[1][:, sl, :].rearrange("p a b -> p (a b)"))
                if generic_gb:
                    nc.vector.tensor_scalar_mul(out=havg[:, sl, :].rearrange("p a b -> p (a b)"),
                                                in0=havg[:, sl, :].rearrange("p a b -> p (a b)"),
                                                scalar1=0.5)
                nc.sync.dma_start(
                    out=h_out_d[s].rearrange("(nb p) h -> p nb h", p=128)[:, sl, :],
                    in_=havg[:, sl, :])
            havg_h = fin.tile([128, L // 128, 128], f16, tag="havg_h", bufs=1, name="havg_h")
            for c in range(NCH):
                sl = slice(NBLK * c, NBLK * (c + 1))
                nc.gpsimd.tensor_copy(havg_h[:, sl, :].rearrange("p a b -> p (a b)"),
                                      havg[:, sl, :].rearrange("p a b -> p (a b)"))
            scores = fin.tile([128, L // 128], f32, tag="scores", name="scores")
            scratch = fin.tile([128, H], f32, tag="scratch", name="scratch")
            for b in range(L // 128):
                nc.vector.scalar_tensor_tensor(out=scratch, in0=havg[:, b, :], scalar=1.0,
                                               in1=Wsbc_sb, op0=ALU.mult, op1=ALU.mult,
                                               accum_out=scores[:, b:b + 1])
            esc = fin.tile([128, L // 128], f16, tag="esc", name="esc")
            nc.scalar.activation(esc, scores, AF.Exp)
            ered = fin.tile([128, 1], f32, tag="ered", name="ered")
            nc.vector.reduce_sum(ered, esc, axis=mybir.AxisListType.X)
            ups = pus.tile([1, 129], f32, name="ups")
            for b in range(L // 128):
                nc.tensor.matmul(ups[:, 0:128], esc[:, b:b + 1], havg_h[:, b, :],
                                 start=(b == 0), stop=(b == L // 128 - 1))
            nc.tensor.matmul(ups[:, 128:129], ered, ones_sb)
            rS = fin.tile([1, 1], f32, tag="rS", name="rS")
            nc.vector.reciprocal(rS, ups[:, 128:129])
            pooled_sb = fin.tile([1, H], f32, tag="pooled_sb", name="pooled_sb")
            nc.vector.tensor_scalar_mul(out=pooled_sb, in0=ups[:, 0:128], scalar1=rS)
            nc.sync.dma_start(out=pooled_d[s:s + 1, :], in_=pooled_sb)

        for p in (pus, ptr, pyy, pmm, fin, stat, work, hB, hA, con):
            p.release()

    nc.compile()
    return nc


def kernel(tokens, Wp, bp, W1, b1, W2, b2, gamma, beta, Ws, bs):
    tokens = np.ascontiguousarray(np.asarray(tokens, np.float32))
    B, L, K = tokens.shape
    BS = B // CORES

    generic_b2 = not np.all(np.asarray(b2) == 0)
    generic_gb = not (np.all(np.asarray(gamma) == 1) and np.all(np.asarray(beta) == 0))

    nc = _build(BS, L, generic_b2=generic_b2, generic_gb=generic_gb)

    base = {
        "Wp": np.ascontiguousarray(np.asarray(Wp).astype(np.float16)),
        "W1h": np.ascontiguousarray(np.asarray(W1).astype(np.float16)),
        "W2h": np.ascontiguousarray(np.asarray(W2).astype(np.float16)),
        "b1T": np.ascontiguousarray(np.asarray(b1, np.float32).T.reshape(H, NL)),
        "bpT": np.ascontiguousarray(np.asarray(bp, np.float32).reshape(H, 1)),
        "Ws_bc": np.ascontiguousarray(np.broadcast_to(np.asarray(Ws, np.float32).reshape(1, H), (H, H))),
        "id128h": np.eye(H, dtype=np.float16),
        "ones128": np.ones((H, 1), np.float32),
    }
    if generic_b2:
        base["b2bc"] = np.ascontiguousarray(
            np.broadcast_to(np.asarray(b2, np.float32).reshape(1, NL * H), (H, NL * H)))
    if generic_gb:
        gb = np.stack([np.asarray(gamma, np.float32), np.asarray(beta, np.float32)])
        base["gb_bc"] = np.ascontiguousarray(
            np.broadcast_to(gb.reshape(1, 2 * NL * H), (H, 2 * NL * H)))

    tokens_h = tokens.astype(np.float16)
    in_maps = [dict(base, tokens_sh=np.ascontiguousarray(tokens_h[i * BS:(i + 1) * BS]))
               for i in range(CORES)]

    from concourse.bass_utils import run_bass_kernel_spmd
    trace = bool(os.environ.get("BCE_TRACE"))
    res = run_bass_kernel_spmd(nc, in_maps, core_ids=list(range(CORES)), trace=trace)
    globals()["_last_results"] = res

    h = np.concatenate([r["h_out"] for r in res.results], axis=0)
    pooled = np.concatenate([r["pooled_out"] for r in res.results], axis=0)
    return (h, pooled)
